# revision 32
# baseline (speedup 1.0000x reference)
"""Trainium2 Bass kernel for nn_MembraneLayer: h = x @ w followed by a
double first-order recurrence over time, producing (syn_rec, mem_rec).

Sharding: data-parallel over batch. 8 cores x 64 batches each.

Radix-2 "weight-folded" design (v3). The DVE serial scan (~2.2 ns/col,
no perf modes) was the baseline bottleneck at 2x1600 scan cols per
(quartet, d_tile) unit. This version halves the scanned columns by
scanning only ODD timesteps and recovering the evens algebraically:

  syn[2k+1] = a^2 syn[2k-1] + (a*h[2k-1] + h[2k])   <- scan over g
  a*syn[2k] = syn[2k+1] - h[2k]                     <- TT sub (fp16 2x)
  V[k] := a*v[2k+1] = b^2 V[k-1] + ab*syn[2k-1] + a*syn[2k]  <- scan g_m
  ab*v[2k]  = V[k] - a*syn[2k]                      <- TT sub
  (v = mem/(1-b); host applies the final per-channel scales)

The pair-combines fold into the PE: host pre-multiplies weight copies
(w*a for odd-t columns, w for even-t), so PE emits the planes
  Po = a*h_odd (x odd cols, leading zero col for t=-1)
  Pe = h_even
at the SAME total matmul column count as the baseline. ACT stages
PSUM->SBUF and computes tmp = (ab)*syn_odd_shifted (per-partition
scale); the g/g_m adds and the two subs are fp16 2x-mode DVE ops
(gpsimd's Q7 software adds measured 3.5-5us/800col strided).

All tiles contiguous [128, 800]; sequence resets via coef=0 at k=0
cols; the shifted read's "syn[-1]=0" comes from a once-per-buffer
zeroed tmp column. Emission is software-pipelined: unit u's syn part
is followed by unit u-1's mem part so ACT's tmp has a full half-unit
to land before the DVE needs g_m.

Outputs per unit are packed in ONE [128, 3200] tile (s_odd | V | Se |
Me) -> single DMA. Host interleaves parities and applies per-channel
scales 1/a, (1-b)/a, (1-b)/(ab).

Measured: 137.5 us (baseline plain-scan kernel: 143.3 us on the same
harness). Restructured v4-v7 variants (fat-row DMAs, 3-queue round-
robin, gpsimd adds) all regressed to 148-210 us: per-queue DMA
dispatch (~25-50 ns/partition-row packet) makes input feed granularity
vs startup latency a hard tradeoff, and gpsimd/ACT offloads stretched
the critical chain. Keeping the proven config.
"""

import os
from contextlib import ExitStack

import numpy as np

import concourse.bass as bass
import concourse.tile as tile
from concourse import bacc, mybir
from concourse import bass_utils

B, T, C, D = 512, 100, 700, 512
NCORES = 8
BC = B // NCORES  # 64 batches per core
NQ = 4  # quartets: 16 batches each
K = 50  # radix-2 pairs per sequence
UO = 16 * K  # cols per unit (800)
OTW = 4 * UO  # packed out tile width (3200)
KT = [(k * 128, min(128, C - k * 128)) for k in range(6)]
F32 = mybir.dt.float32
FP16 = mybir.dt.float16
MULT = mybir.AluOpType.mult
ADD = mybir.AluOpType.add
COPY = mybir.ActivationFunctionType.Copy

MODE = "radix2-wfold-v3"
LAST_RESULT = None
_cache = {}


def _build():
    key = ("nc",)
    if key in _cache:
        return _cache[key]
    nc = bacc.Bacc("TRN2", target_bir_lowering=False, debug=False)

    xe_d = nc.dram_tensor("xe16", [C, BC * K], FP16, kind="ExternalInput").ap()
    xo_d = nc.dram_tensor("xo16", [C, BC * K], FP16, kind="ExternalInput").ap()
    w_d = nc.dram_tensor("w16", [C, D], FP16, kind="ExternalInput").ap()
    wa_d = nc.dram_tensor("wa16", [C, D], FP16, kind="ExternalInput").ap()
    # combined per-dtile coefs: [acoef 800 | bcoef 800 | abscale 1]
    cf_d = nc.dram_tensor("coefs", [4, 128, 2 * UO + 1], F32, kind="ExternalInput").ap()
    out_d = nc.dram_tensor("out", [D, NQ * OTW], FP16, kind="ExternalOutput").ap()

    with tile.TileContext(nc) as tc:
        with ExitStack() as ctx:
            cpool = ctx.enter_context(tc.tile_pool(name="consts", bufs=1))
            warm_sb = cpool.tile([128, 512], FP16, name="warm", tag="warm")
            nc.gpsimd.memset(warm_sb[:], 0.0)
            # weights on the gpsimd (SWDGE) queue; coefs on the scalar queue
            # (ACT is idle until the first staging) so the Sync queue leads
            # with the first x tiles and gpsimd's queue stays shallow
            w_tiles, wa_tiles = [], []
            for k, (r0_, rk) in enumerate(KT):
                wt = cpool.tile([128, D], FP16, name=f"w{k}", tag=f"w{k}")
                nc.gpsimd.dma_start(wt[:rk, :], w_d[r0_ : r0_ + rk, :])
                w_tiles.append(wt)
                wat = cpool.tile([128, D], FP16, name=f"wa{k}", tag=f"wa{k}")
                nc.gpsimd.dma_start(wat[:rk, :], wa_d[r0_ : r0_ + rk, :])
                wa_tiles.append(wat)
            cf_t = []
            for di in range(4):
                t_ = cpool.tile([128, 2 * UO + 1], F32, name=f"cf{di}", tag=f"cf{di}")
                nc.scalar.dma_start(t_[:], cf_d[di])
                cf_t.append(t_)

            xp = ctx.enter_context(tc.tile_pool(name="xp", bufs=2))
            pp = ctx.enter_context(tc.tile_pool(name="pp", bufs=2, space="PSUM"))
            sp = ctx.enter_context(tc.tile_pool(name="sp", bufs=3))
            gp = ctx.enter_context(tc.tile_pool(name="gp", bufs=3))
            op = ctx.enter_context(tc.tile_pool(name="op", bufs=3))

            # PE warmup: enough to trigger the p-state ramp without
            # overshooting the initial DMA wait
            warm_ps = pp.tile([128, 2048], F32, tag="ps", name="warm_ps")
            for _ in range(10):
                nc.tensor.matmul(
                    warm_ps[:, 0:384], warm_sb[:, 0:128], warm_sb[:, 0:384],
                    start=True, stop=True,
                )

            state = {}

            def syn_part(u):
                q, di = u
                dsl = slice(di * 128, (di + 1) * 128)
                xe_ts, xo_ts = state["x"]

                ps = pp.tile([128, 2048], F32, tag="ps", name=f"ps_{q}_{di}")
                po = ps[:, 0:UO]
                pe = ps[:, 1024 : 1024 + UO]
                # matmul outputs may not cross PSUM bank boundaries
                for k, (r0_, rk) in enumerate(KT):
                    for c0, c1 in ((0, 512), (512, UO)):
                        nc.tensor.matmul(
                            po[:, c0:c1], wa_tiles[k][:rk, dsl],
                            xo_ts[k][:rk, c0:c1],
                            start=(k == 0), stop=(k == 5),
                        )
                for k, (r0_, rk) in enumerate(KT):
                    for c0, c1 in ((0, 512), (512, UO)):
                        nc.tensor.matmul(
                            pe[:, c0:c1], w_tiles[k][:rk, dsl],
                            xe_ts[k][:rk, c0:c1],
                            start=(k == 0), stop=(k == 5),
                        )

                po16 = sp.tile([128, UO], FP16, tag="po16", name=f"po16_{q}_{di}")
                nc.scalar.activation(po16[:], po, COPY)
                pe16 = sp.tile([128, UO], FP16, tag="pe16", name=f"pe16_{q}_{di}")
                nc.scalar.activation(pe16[:], pe, COPY)

                ot = op.tile([128, OTW], FP16, tag="ot", name=f"ot_{q}_{di}")
                s_odd = ot[:, 0:UO]
                Se = ot[:, 2 * UO : 3 * UO]

                g = gp.tile([128, UO], FP16, tag="g", name=f"g_{q}_{di}")
                # g-add stays on DVE: offloading it to gpsimd A/B-tested
                # SLOWER overall (DVE busy rose 89->101us; cross-engine hops
                # inflate DVE per-op overhead and add mid-run stalls)
                nc.vector.tensor_add(g[:], po16[:], pe16[:])
                nc.vector.tensor_tensor_scan(
                    s_odd, cf_t[di][:, 0:UO], g[:], 0.0, MULT, ADD
                )
                nc.vector.tensor_sub(Se, s_odd, pe16[:])
                state[u] = (ot, dsl)

            def mem_part(u, mcall):
                q, di = u
                ot, dsl = state.pop(u)
                s_odd = ot[:, 0:UO]
                V = ot[:, UO : 2 * UO]
                Se = ot[:, 2 * UO : 3 * UO]
                Me = ot[:, 3 * UO : OTW]
                s3 = s_odd.rearrange("p (s c) -> p s c", c=K)

                # tmp = (a*b) * syn[2k-1]; col k=0 per seq must be 0 -
                # zeroed once per pool buffer (ACT never writes col 0)
                tmp = sp.tile([128, UO], FP16, tag="tmp", name=f"tmp_{q}_{di}")
                t3 = tmp[:].rearrange("p (s c) -> p s c", c=K)
                if mcall < 3:
                    nc.gpsimd.memset(t3[:, :, 0:1], 0.0)
                nc.scalar.activation(
                    t3[:, :, 1:K], s3[:, :, 0 : K - 1], COPY,
                    scale=cf_t[di][:, 2 * UO : 2 * UO + 1],
                )

                gm = gp.tile([128, UO], FP16, tag="gm", name=f"gm_{q}_{di}")
                nc.vector.tensor_add(gm[:], tmp[:], Se)
                nc.vector.tensor_tensor_scan(
                    V, cf_t[di][:, UO : 2 * UO], gm[:], 0.0, MULT, ADD
                )
                # all outs on the scalar HWDGE queue (splitting them onto the
                # gpsimd queue A/B-tested 3us SLOWER: SWDGE issuance on the
                # gpsimd engine interferes with the compute chain).
                # Last two units: store the already-final streams before the
                # Me sub so the post-final-op drain is one 205KB transfer
                # instead of 819KB (~4us off the tail).
                if mcall >= 12:
                    nc.scalar.dma_start(
                        out_d[dsl, q * OTW : q * OTW + 3 * UO], ot[:, 0 : 3 * UO]
                    )
                    nc.vector.tensor_sub(Me, V, Se)
                    nc.scalar.dma_start(
                        out_d[dsl, q * OTW + 3 * UO : (q + 1) * OTW],
                        ot[:, 3 * UO : OTW],
                    )
                else:
                    nc.vector.tensor_sub(Me, V, Se)
                    nc.scalar.dma_start(
                        out_d[dsl, q * OTW : (q + 1) * OTW], ot[:]
                    )

            units = [(q, di) for q in range(NQ) for di in range(4)]
            prev = None
            mcall = 0
            for u in units:
                q, di = u
                if di == 0:
                    xe_ts, xo_ts = [], []
                    qc0 = q * UO
                    for k, (r0_, rk) in enumerate(KT):
                        te = xp.tile([128, UO], FP16, tag=f"xe{k}", name=f"xe{k}_{q}")
                        nc.sync.dma_start(
                            te[:rk, :], xe_d[r0_ : r0_ + rk, qc0 : qc0 + UO]
                        )
                        xe_ts.append(te)
                        to = xp.tile([128, UO], FP16, tag=f"xo{k}", name=f"xo{k}_{q}")
                        nc.sync.dma_start(
                            to[:rk, :], xo_d[r0_ : r0_ + rk, qc0 : qc0 + UO]
                        )
                        xo_ts.append(to)
                    state["x"] = (xe_ts, xo_ts)
                syn_part(u)
                if prev is not None:
                    mem_part(prev, mcall)
                    mcall += 1
                prev = u
            mem_part(prev, mcall)

    nc.compile()
    _cache[key] = nc
    return nc


def kernel(inputs, w, alpha, beta):
    global LAST_RESULT
    inputs = np.asarray(inputs, dtype=np.float32)
    w = np.asarray(w, dtype=np.float32)
    alpha = np.asarray(alpha, dtype=np.float32).reshape(-1)
    beta = np.asarray(beta, dtype=np.float32).reshape(-1)

    nc = _build()

    def coef(sq):
        c = np.broadcast_to(sq.reshape(4, 128, 1), (4, 128, UO)).astype(np.float32).copy()
        c3 = c.reshape(4, 128, 16, K)
        c3[:, :, :, 0] = 0.0
        return c

    coefs = np.concatenate(
        [
            coef(alpha * alpha),
            coef(beta * beta),
            (alpha * beta).reshape(4, 128, 1).astype(np.float32),
        ],
        axis=2,
    )
    w16 = w.astype(np.float16)
    wa16 = (w * alpha.reshape(1, D)).astype(np.float16)

    in_maps = []
    for c in range(NCORES):
        xc = inputs[c * BC : (c + 1) * BC]  # [64, 100, 700]
        xe = xc[:, 0::2, :]  # [64, 50, 700] (t = 0,2,..,98)
        xo = np.zeros((BC, K, C), dtype=np.float32)
        xo[:, 1:, :] = xc[:, 1:98:2, :]  # t = 1,3,..,97 -> k=1..49
        xe16 = xe.reshape(BC * K, C).T.astype(np.float16).copy()
        xo16 = xo.reshape(BC * K, C).T.astype(np.float16).copy()
        in_maps.append(
            {
                "xe16": xe16,
                "xo16": xo16,
                "w16": w16,
                "wa16": wa16,
                "coefs": coefs,
            }
        )

    run_kwargs = {}
    if os.environ.get("MEMBRANE_TRACE_DIR"):
        run_kwargs["tmpdir"] = os.environ["MEMBRANE_TRACE_DIR"]
    res = bass_utils.run_bass_kernel_spmd(
        nc, in_maps, core_ids=list(range(NCORES)), **run_kwargs
    )
    LAST_RESULT = res

    inv_a = (1.0 / alpha).reshape(1, 1, D)
    mo_sc = ((1.0 - beta) / alpha).reshape(1, 1, D)
    me_sc = ((1.0 - beta) / (alpha * beta)).reshape(1, 1, D)

    syn_full = np.empty((B, T, D), dtype=np.float32)
    mem_full = np.empty((B, T, D), dtype=np.float32)
    for c in range(NCORES):
        r = res.results[c]["out"].astype(np.float32)  # [512, NQ*OTW]
        for q in range(NQ):
            blk = r[:, q * OTW : (q + 1) * OTW]
            so = blk[:, 0:UO].reshape(D, 16, K)  # t=2k+1
            V = blk[:, UO : 2 * UO].reshape(D, 16, K)
            Sev = blk[:, 2 * UO : 3 * UO].reshape(D, 16, K)
            Mev = blk[:, 3 * UO : OTW].reshape(D, 16, K)
            b0 = c * BC + q * 16
            syn_full[b0 : b0 + 16, 1::2, :] = so.transpose(1, 2, 0)
            syn_full[b0 : b0 + 16, 0::2, :] = Sev.transpose(1, 2, 0) * inv_a
            mem_full[b0 : b0 + 16, 1::2, :] = V.transpose(1, 2, 0) * mo_sc
            mem_full[b0 : b0 + 16, 0::2, :] = Mev.transpose(1, 2, 0) * me_sc
    return (syn_full, mem_full)


# revision 33
# speedup vs baseline: 1.0222x; 1.0222x over previous
"""Trainium2 Bass kernel for nn_MembraneLayer: h = x @ w followed by a
double first-order recurrence over time, producing (syn_rec, mem_rec).

Sharding: data-parallel over batch. 8 cores x 64 batches each.

Radix-2 "weight-folded" design (v3). The DVE serial scan (~2.2 ns/col,
no perf modes) was the baseline bottleneck at 2x1600 scan cols per
(quartet, d_tile) unit. This version halves the scanned columns by
scanning only ODD timesteps and recovering the evens algebraically:

  syn[2k+1] = a^2 syn[2k-1] + (a*h[2k-1] + h[2k])   <- scan over g
  a*syn[2k] = syn[2k+1] - h[2k]                     <- TT sub (fp16 2x)
  V[k] := a*v[2k+1] = b^2 V[k-1] + ab*syn[2k-1] + a*syn[2k]  <- scan g_m
  ab*v[2k]  = V[k] - a*syn[2k]                      <- TT sub
  (v = mem/(1-b); host applies the final per-channel scales)

The pair-combines fold into the PE: host pre-multiplies weight copies
(w*a for odd-t columns, w for even-t), so PE emits the planes
  Po = a*h_odd (x odd cols, leading zero col for t=-1)
  Pe = h_even
at the SAME total matmul column count as the baseline. ACT stages
PSUM->SBUF and computes tmp = (ab)*syn_odd_shifted (per-partition
scale); the g/g_m adds and the two subs are fp16 2x-mode DVE ops
(gpsimd's Q7 software adds measured 3.5-5us/800col strided).

All tiles contiguous [128, 800]; sequence resets via coef=0 at k=0
cols; the shifted read's "syn[-1]=0" comes from a once-per-buffer
zeroed tmp column. Emission is software-pipelined: unit u's syn part
is followed by unit u-1's mem part so ACT's tmp has a full half-unit
to land before the DVE needs g_m.

Outputs per unit are packed in ONE [128, 3200] tile (s_odd | V | Se |
Me) -> single DMA. Host interleaves parities and applies per-channel
scales 1/a, (1-b)/a, (1-b)/(ab).

Measured: 137.5 us (baseline plain-scan kernel: 143.3 us on the same
harness). Restructured v4-v7 variants (fat-row DMAs, 3-queue round-
robin, gpsimd adds) all regressed to 148-210 us: per-queue DMA
dispatch (~25-50 ns/partition-row packet) makes input feed granularity
vs startup latency a hard tradeoff, and gpsimd/ACT offloads stretched
the critical chain. Keeping the proven config.
"""

import os
from contextlib import ExitStack

import numpy as np

import concourse.bass as bass
import concourse.tile as tile
from concourse import bacc, mybir
from concourse import bass_utils

B, T, C, D = 512, 100, 700, 512
NCORES = 8
BC = B // NCORES  # 64 batches per core
NQ = 4  # quartets: 16 batches each
K = 50  # radix-2 pairs per sequence
UO = 16 * K  # cols per unit (800)
OTW = 4 * UO  # packed out tile width (3200)
KT = [(k * 128, min(128, C - k * 128)) for k in range(6)]
F32 = mybir.dt.float32
FP16 = mybir.dt.float16
MULT = mybir.AluOpType.mult
ADD = mybir.AluOpType.add
COPY = mybir.ActivationFunctionType.Copy

MODE = "radix2-wfold-v3"
LAST_RESULT = None
_cache = {}


def _build():
    key = ("nc",)
    if key in _cache:
        return _cache[key]
    nc = bacc.Bacc("TRN2", target_bir_lowering=False, debug=False)

    xe_d = nc.dram_tensor("xe16", [C, BC * K], FP16, kind="ExternalInput").ap()
    xo_d = nc.dram_tensor("xo16", [C, BC * K], FP16, kind="ExternalInput").ap()
    w_d = nc.dram_tensor("w16", [C, D], FP16, kind="ExternalInput").ap()
    wa_d = nc.dram_tensor("wa16", [C, D], FP16, kind="ExternalInput").ap()
    # combined per-dtile coefs: [acoef 800 | bcoef 800 | abscale 1]
    cf_d = nc.dram_tensor("coefs", [4, 128, 2 * UO + 1], F32, kind="ExternalInput").ap()
    out_d = nc.dram_tensor("out", [D, NQ * OTW], FP16, kind="ExternalOutput").ap()

    with tile.TileContext(nc) as tc:
        with ExitStack() as ctx:
            cpool = ctx.enter_context(tc.tile_pool(name="consts", bufs=1))
            warm_sb = cpool.tile([128, 512], FP16, name="warm", tag="warm")
            nc.gpsimd.memset(warm_sb[:], 0.0)
            # weights on the gpsimd (SWDGE) queue; coefs on the scalar queue
            # (ACT is idle until the first staging) so the Sync queue leads
            # with the first x tiles and gpsimd's queue stays shallow
            w_tiles, wa_tiles = [], []
            for k, (r0_, rk) in enumerate(KT):
                wt = cpool.tile([128, D], FP16, name=f"w{k}", tag=f"w{k}")
                nc.gpsimd.dma_start(wt[:rk, :], w_d[r0_ : r0_ + rk, :])
                w_tiles.append(wt)
                wat = cpool.tile([128, D], FP16, name=f"wa{k}", tag=f"wa{k}")
                nc.gpsimd.dma_start(wat[:rk, :], wa_d[r0_ : r0_ + rk, :])
                wa_tiles.append(wat)
            cf_t = []
            for di in range(4):
                t_ = cpool.tile([128, 2 * UO + 1], F32, name=f"cf{di}", tag=f"cf{di}")
                nc.scalar.dma_start(t_[:], cf_d[di])
                cf_t.append(t_)

            xp = ctx.enter_context(tc.tile_pool(name="xp", bufs=2))
            pp = ctx.enter_context(tc.tile_pool(name="pp", bufs=2, space="PSUM"))
            sp = ctx.enter_context(tc.tile_pool(name="sp", bufs=3))
            gp = ctx.enter_context(tc.tile_pool(name="gp", bufs=3))
            op = ctx.enter_context(tc.tile_pool(name="op", bufs=3))

            # PE warmup: enough to trigger the p-state ramp without
            # overshooting the initial DMA wait
            warm_ps = pp.tile([128, 2048], F32, tag="ps", name="warm_ps")
            for _ in range(10):
                nc.tensor.matmul(
                    warm_ps[:, 0:384], warm_sb[:, 0:128], warm_sb[:, 0:384],
                    start=True, stop=True,
                )

            state = {}

            def syn_part(u):
                q, di = u
                dsl = slice(di * 128, (di + 1) * 128)
                xe_ts, xo_ts = state["x"]

                ps = pp.tile([128, 2048], F32, tag="ps", name=f"ps_{q}_{di}")
                po = ps[:, 0:UO]
                pe = ps[:, 1024 : 1024 + UO]
                # matmul outputs may not cross PSUM bank boundaries
                for k, (r0_, rk) in enumerate(KT):
                    for c0, c1 in ((0, 512), (512, UO)):
                        nc.tensor.matmul(
                            po[:, c0:c1], wa_tiles[k][:rk, dsl],
                            xo_ts[k][:rk, c0:c1],
                            start=(k == 0), stop=(k == 5),
                        )
                for k, (r0_, rk) in enumerate(KT):
                    for c0, c1 in ((0, 512), (512, UO)):
                        nc.tensor.matmul(
                            pe[:, c0:c1], w_tiles[k][:rk, dsl],
                            xe_ts[k][:rk, c0:c1],
                            start=(k == 0), stop=(k == 5),
                        )

                po16 = sp.tile([128, UO], FP16, tag="po16", name=f"po16_{q}_{di}")
                nc.scalar.activation(po16[:], po, COPY)
                pe16 = sp.tile([128, UO], FP16, tag="pe16", name=f"pe16_{q}_{di}")
                nc.scalar.activation(pe16[:], pe, COPY)

                ot = op.tile([128, OTW], FP16, tag="ot", name=f"ot_{q}_{di}")
                s_odd = ot[:, 0:UO]
                Se = ot[:, 2 * UO : 3 * UO]

                g = gp.tile([128, UO], FP16, tag="g", name=f"g_{q}_{di}")
                # g-add stays on DVE: offloading it to gpsimd A/B-tested
                # SLOWER overall (DVE busy rose 89->101us; cross-engine hops
                # inflate DVE per-op overhead and add mid-run stalls)
                nc.vector.tensor_add(g[:], po16[:], pe16[:])
                nc.vector.tensor_tensor_scan(
                    s_odd, cf_t[di][:, 0:UO], g[:], 0.0, MULT, ADD
                )
                nc.vector.tensor_sub(Se, s_odd, pe16[:])
                state[u] = (ot, dsl)

            def mem_part(u, mcall):
                q, di = u
                ot, dsl = state.pop(u)
                s_odd = ot[:, 0:UO]
                V = ot[:, UO : 2 * UO]
                Se = ot[:, 2 * UO : 3 * UO]
                Me = ot[:, 3 * UO : OTW]
                s3 = s_odd.rearrange("p (s c) -> p s c", c=K)

                # tmp = (a*b) * syn[2k-1]; col k=0 per seq must be 0 -
                # zeroed once per pool buffer (ACT never writes col 0)
                tmp = sp.tile([128, UO], FP16, tag="tmp", name=f"tmp_{q}_{di}")
                t3 = tmp[:].rearrange("p (s c) -> p s c", c=K)
                if mcall < 3:
                    nc.gpsimd.memset(t3[:, :, 0:1], 0.0)
                nc.scalar.activation(
                    t3[:, :, 1:K], s3[:, :, 0 : K - 1], COPY,
                    scale=cf_t[di][:, 2 * UO : 2 * UO + 1],
                )

                gm = gp.tile([128, UO], FP16, tag="gm", name=f"gm_{q}_{di}")
                nc.vector.tensor_add(gm[:], tmp[:], Se)
                nc.vector.tensor_tensor_scan(
                    V, cf_t[di][:, UO : 2 * UO], gm[:], 0.0, MULT, ADD
                )
                # all outs on the scalar HWDGE queue (splitting them onto the
                # gpsimd queue A/B-tested 3us SLOWER: SWDGE issuance on the
                # gpsimd engine interferes with the compute chain).
                # Last two units: store the already-final streams before the
                # Me sub so the post-final-op drain is one 205KB transfer
                # instead of 819KB (~4us off the tail).
                if mcall >= 14:
                    nc.scalar.dma_start(
                        out_d[dsl, q * OTW : q * OTW + 3 * UO], ot[:, 0 : 3 * UO]
                    )
                    nc.vector.tensor_sub(Me, V, Se)
                    nc.scalar.dma_start(
                        out_d[dsl, q * OTW + 3 * UO : (q + 1) * OTW],
                        ot[:, 3 * UO : OTW],
                    )
                else:
                    nc.vector.tensor_sub(Me, V, Se)
                    nc.scalar.dma_start(
                        out_d[dsl, q * OTW : (q + 1) * OTW], ot[:]
                    )

            units = [(q, di) for q in range(NQ) for di in range(4)]
            prev = None
            mcall = 0
            for u in units:
                q, di = u
                if di == 0:
                    xe_ts, xo_ts = [], []
                    qc0 = q * UO
                    for k, (r0_, rk) in enumerate(KT):
                        te = xp.tile([128, UO], FP16, tag=f"xe{k}", name=f"xe{k}_{q}")
                        nc.sync.dma_start(
                            te[:rk, :], xe_d[r0_ : r0_ + rk, qc0 : qc0 + UO]
                        )
                        xe_ts.append(te)
                        to = xp.tile([128, UO], FP16, tag=f"xo{k}", name=f"xo{k}_{q}")
                        nc.sync.dma_start(
                            to[:rk, :], xo_d[r0_ : r0_ + rk, qc0 : qc0 + UO]
                        )
                        xo_ts.append(to)
                    state["x"] = (xe_ts, xo_ts)
                syn_part(u)
                if prev is not None:
                    mem_part(prev, mcall)
                    mcall += 1
                prev = u
            mem_part(prev, mcall)

    nc.compile()
    _cache[key] = nc
    return nc


def kernel(inputs, w, alpha, beta):
    global LAST_RESULT
    inputs = np.asarray(inputs, dtype=np.float32)
    w = np.asarray(w, dtype=np.float32)
    alpha = np.asarray(alpha, dtype=np.float32).reshape(-1)
    beta = np.asarray(beta, dtype=np.float32).reshape(-1)

    nc = _build()

    def coef(sq):
        c = np.broadcast_to(sq.reshape(4, 128, 1), (4, 128, UO)).astype(np.float32).copy()
        c3 = c.reshape(4, 128, 16, K)
        c3[:, :, :, 0] = 0.0
        return c

    coefs = np.concatenate(
        [
            coef(alpha * alpha),
            coef(beta * beta),
            (alpha * beta).reshape(4, 128, 1).astype(np.float32),
        ],
        axis=2,
    )
    w16 = w.astype(np.float16)
    wa16 = (w * alpha.reshape(1, D)).astype(np.float16)

    in_maps = []
    for c in range(NCORES):
        xc = inputs[c * BC : (c + 1) * BC]  # [64, 100, 700]
        xe = xc[:, 0::2, :]  # [64, 50, 700] (t = 0,2,..,98)
        xo = np.zeros((BC, K, C), dtype=np.float32)
        xo[:, 1:, :] = xc[:, 1:98:2, :]  # t = 1,3,..,97 -> k=1..49
        xe16 = xe.reshape(BC * K, C).T.astype(np.float16).copy()
        xo16 = xo.reshape(BC * K, C).T.astype(np.float16).copy()
        in_maps.append(
            {
                "xe16": xe16,
                "xo16": xo16,
                "w16": w16,
                "wa16": wa16,
                "coefs": coefs,
            }
        )

    run_kwargs = {}
    if os.environ.get("MEMBRANE_TRACE_DIR"):
        run_kwargs["tmpdir"] = os.environ["MEMBRANE_TRACE_DIR"]
    res = bass_utils.run_bass_kernel_spmd(
        nc, in_maps, core_ids=list(range(NCORES)), **run_kwargs
    )
    LAST_RESULT = res

    inv_a = (1.0 / alpha).reshape(1, 1, D)
    mo_sc = ((1.0 - beta) / alpha).reshape(1, 1, D)
    me_sc = ((1.0 - beta) / (alpha * beta)).reshape(1, 1, D)

    syn_full = np.empty((B, T, D), dtype=np.float32)
    mem_full = np.empty((B, T, D), dtype=np.float32)
    for c in range(NCORES):
        r = res.results[c]["out"].astype(np.float32)  # [512, NQ*OTW]
        for q in range(NQ):
            blk = r[:, q * OTW : (q + 1) * OTW]
            so = blk[:, 0:UO].reshape(D, 16, K)  # t=2k+1
            V = blk[:, UO : 2 * UO].reshape(D, 16, K)
            Sev = blk[:, 2 * UO : 3 * UO].reshape(D, 16, K)
            Mev = blk[:, 3 * UO : OTW].reshape(D, 16, K)
            b0 = c * BC + q * 16
            syn_full[b0 : b0 + 16, 1::2, :] = so.transpose(1, 2, 0)
            syn_full[b0 : b0 + 16, 0::2, :] = Sev.transpose(1, 2, 0) * inv_a
            mem_full[b0 : b0 + 16, 1::2, :] = V.transpose(1, 2, 0) * mo_sc
            mem_full[b0 : b0 + 16, 0::2, :] = Mev.transpose(1, 2, 0) * me_sc
    return (syn_full, mem_full)


# revision 40
# speedup vs baseline: 1.0551x; 1.0321x over previous
"""Trainium2 Bass kernel for nn_MembraneLayer: h = x @ w followed by a
double first-order recurrence over time, producing (syn_rec, mem_rec).

Sharding: data-parallel over batch. 8 cores x 64 batches each.

Radix-2 "weight-folded" design (v3). The DVE serial scan (~2.2 ns/col,
no perf modes) was the baseline bottleneck at 2x1600 scan cols per
(quartet, d_tile) unit. This version halves the scanned columns by
scanning only ODD timesteps and recovering the evens algebraically:

  syn[2k+1] = a^2 syn[2k-1] + (a*h[2k-1] + h[2k])   <- scan over g
  a*syn[2k] = syn[2k+1] - h[2k]                     <- TT sub (fp16 2x)
  V[k] := a*v[2k+1] = b^2 V[k-1] + ab*syn[2k-1] + a*syn[2k]  <- scan g_m
  ab*v[2k]  = V[k] - a*syn[2k]                      <- TT sub
  (v = mem/(1-b); host applies the final per-channel scales)

The pair-combines fold into the PE: host pre-multiplies weight copies
(w*a for odd-t columns, w for even-t), so PE emits the planes
  Po = a*h_odd (x odd cols, leading zero col for t=-1)
  Pe = h_even
at the SAME total matmul column count as the baseline. ACT stages
PSUM->SBUF and computes tmp = (ab)*syn_odd_shifted (per-partition
scale); the g/g_m adds and the two subs are fp16 2x-mode DVE ops
(gpsimd's Q7 software adds measured 3.5-5us/800col strided).

All tiles contiguous [128, 800]; sequence resets via coef=0 at k=0
cols; the shifted read's "syn[-1]=0" comes from a once-per-buffer
zeroed tmp column. Emission is software-pipelined: unit u's syn part
is followed by unit u-1's mem part so ACT's tmp has a full half-unit
to land before the DVE needs g_m.

Outputs per unit are packed in ONE [128, 3200] tile (s_odd | V | Se |
Me) -> single DMA. Host interleaves parities and applies per-channel
scales 1/a, (1-b)/a, (1-b)/(ab).

Measured: 137.5 us (baseline plain-scan kernel: 143.3 us on the same
harness). Restructured v4-v7 variants (fat-row DMAs, 3-queue round-
robin, gpsimd adds) all regressed to 148-210 us: per-queue DMA
dispatch (~25-50 ns/partition-row packet) makes input feed granularity
vs startup latency a hard tradeoff, and gpsimd/ACT offloads stretched
the critical chain. Keeping the proven config.
"""

import os
from contextlib import ExitStack

import numpy as np

import concourse.bass as bass
import concourse.tile as tile
from concourse import bacc, mybir
from concourse import bass_utils

B, T, C, D = 512, 100, 700, 512
NCORES = 8
BC = B // NCORES  # 64 batches per core
NQ = 4  # quartets: 16 batches each
K = 50  # radix-2 pairs per sequence
UO = 16 * K  # cols per unit (800)
OTW = 4 * UO  # packed out tile width (3200)
KT = [(k * 128, min(128, C - k * 128)) for k in range(6)]
F32 = mybir.dt.float32
FP16 = mybir.dt.float16
MULT = mybir.AluOpType.mult
ADD = mybir.AluOpType.add
COPY = mybir.ActivationFunctionType.Copy

MODE = "radix2-wfold-v3"
LAST_RESULT = None
_cache = {}


def _build():
    key = ("nc",)
    if key in _cache:
        return _cache[key]
    nc = bacc.Bacc("TRN2", target_bir_lowering=False, debug=False)

    xe_d = nc.dram_tensor("xe16", [C, BC * K], FP16, kind="ExternalInput").ap()
    xo_d = nc.dram_tensor("xo16", [C, BC * K], FP16, kind="ExternalInput").ap()
    # weights packed [w 6x512 | w*a 6x512]: ktile k at col k*512 within a half
    w_d = nc.dram_tensor("w16", [128, 12 * D], FP16, kind="ExternalInput").ap()
    # combined per-dtile coefs: [acoef 800 | bcoef 800 | abscale 1]
    cf_d = nc.dram_tensor("coefs", [4, 128, 2 * UO + 1], F32, kind="ExternalInput").ap()
    out_d = nc.dram_tensor("out", [D, NQ * OTW], FP16, kind="ExternalOutput").ap()

    with tile.TileContext(nc) as tc:
        with ExitStack() as ctx:
            cpool = ctx.enter_context(tc.tile_pool(name="consts", bufs=1))
            warm_sb = cpool.tile([128, 512], FP16, name="warm", tag="warm")
            nc.gpsimd.memset(warm_sb[:], 0.0)
            # startup queue balance (first-unit inputs = w 1.43MB + x-q0
            # 2.24MB + cf0 0.82MB across 3 queues): packed w as 2 fat DMAs
            # on scalar, xe-q0 on sync, xo-q0 on gpsimd, coefs scattered
            # after their queue's critical load, in need order
            w_all = cpool.tile([128, 12 * D], FP16, name="w_all", tag="w_all")
            nc.scalar.dma_start(w_all[:, 0 : 6 * D], w_d[:, 0 : 6 * D])
            nc.scalar.dma_start(w_all[:, 6 * D : 12 * D], w_d[:, 6 * D : 12 * D])
            cf_t = []
            for di in range(4):
                t_ = cpool.tile([128, 2 * UO + 1], F32, name=f"cf{di}", tag=f"cf{di}")
                cf_t.append(t_)
            nc.scalar.dma_start(cf_t[0][:], cf_d[0])
            nc.scalar.dma_start(cf_t[3][:], cf_d[3])

            xp = ctx.enter_context(tc.tile_pool(name="xp", bufs=2))
            pp = ctx.enter_context(tc.tile_pool(name="pp", bufs=2, space="PSUM"))
            sp = ctx.enter_context(tc.tile_pool(name="sp", bufs=3))
            gp = ctx.enter_context(tc.tile_pool(name="gp", bufs=3))
            op = ctx.enter_context(tc.tile_pool(name="op", bufs=3))

            # PE warmup: enough to trigger the p-state ramp without
            # overshooting the initial DMA wait
            warm_ps = pp.tile([128, 2048], F32, tag="ps", name="warm_ps")
            for _ in range(10):
                nc.tensor.matmul(
                    warm_ps[:, 0:384], warm_sb[:, 0:128], warm_sb[:, 0:384],
                    start=True, stop=True,
                )

            state = {}

            def syn_part(u):
                q, di = u
                dsl = slice(di * 128, (di + 1) * 128)
                xe_ts, xo_ts = state["x"]

                ps = pp.tile([128, 2048], F32, tag="ps", name=f"ps_{q}_{di}")
                po = ps[:, 0:UO]
                pe = ps[:, 1024 : 1024 + UO]
                # matmul outputs may not cross PSUM bank boundaries
                for k, (r0_, rk) in enumerate(KT):
                    wo = 6 * D + k * D + di * 128
                    for c0, c1 in ((0, 512), (512, UO)):
                        nc.tensor.matmul(
                            po[:, c0:c1], w_all[:rk, wo : wo + 128],
                            xo_ts[k][:rk, c0:c1],
                            start=(k == 0), stop=(k == 5),
                        )
                for k, (r0_, rk) in enumerate(KT):
                    we = k * D + di * 128
                    for c0, c1 in ((0, 512), (512, UO)):
                        nc.tensor.matmul(
                            pe[:, c0:c1], w_all[:rk, we : we + 128],
                            xe_ts[k][:rk, c0:c1],
                            start=(k == 0), stop=(k == 5),
                        )

                po16 = sp.tile([128, UO], FP16, tag="po16", name=f"po16_{q}_{di}")
                nc.scalar.activation(po16[:], po, COPY)
                pe16 = sp.tile([128, UO], FP16, tag="pe16", name=f"pe16_{q}_{di}")
                nc.scalar.activation(pe16[:], pe, COPY)

                ot = op.tile([128, OTW], FP16, tag="ot", name=f"ot_{q}_{di}")
                s_odd = ot[:, 0:UO]
                Se = ot[:, 2 * UO : 3 * UO]

                g = gp.tile([128, UO], FP16, tag="g", name=f"g_{q}_{di}")
                # g-add stays on DVE: offloading it to gpsimd A/B-tested
                # SLOWER overall (DVE busy rose 89->101us; cross-engine hops
                # inflate DVE per-op overhead and add mid-run stalls)
                nc.vector.tensor_add(g[:], po16[:], pe16[:])
                nc.vector.tensor_tensor_scan(
                    s_odd, cf_t[di][:, 0:UO], g[:], 0.0, MULT, ADD
                )
                nc.vector.tensor_sub(Se, s_odd, pe16[:])
                state[u] = (ot, dsl)

            def mem_part(u, mcall):
                q, di = u
                ot, dsl = state.pop(u)
                s_odd = ot[:, 0:UO]
                V = ot[:, UO : 2 * UO]
                Se = ot[:, 2 * UO : 3 * UO]
                Me = ot[:, 3 * UO : OTW]
                s3 = s_odd.rearrange("p (s c) -> p s c", c=K)

                # tmp = (a*b) * syn[2k-1]; col k=0 per seq must be 0 -
                # zeroed once per pool buffer (ACT never writes col 0)
                tmp = sp.tile([128, UO], FP16, tag="tmp", name=f"tmp_{q}_{di}")
                t3 = tmp[:].rearrange("p (s c) -> p s c", c=K)
                if mcall < 3:
                    nc.gpsimd.memset(t3[:, :, 0:1], 0.0)
                nc.scalar.activation(
                    t3[:, :, 1:K], s3[:, :, 0 : K - 1], COPY,
                    scale=cf_t[di][:, 2 * UO : 2 * UO + 1],
                )

                gm = gp.tile([128, UO], FP16, tag="gm", name=f"gm_{q}_{di}")
                nc.vector.tensor_add(gm[:], tmp[:], Se)
                nc.vector.tensor_tensor_scan(
                    V, cf_t[di][:, UO : 2 * UO], gm[:], 0.0, MULT, ADD
                )
                # all outs on the scalar HWDGE queue (splitting them onto the
                # gpsimd queue A/B-tested 3us SLOWER: SWDGE issuance on the
                # gpsimd engine interferes with the compute chain).
                # Last two units: store the already-final streams before the
                # Me sub so the post-final-op drain is one 205KB transfer
                # instead of 819KB (~4us off the tail).
                if mcall >= 14:
                    nc.scalar.dma_start(
                        out_d[dsl, q * OTW : q * OTW + 3 * UO], ot[:, 0 : 3 * UO]
                    )
                    nc.vector.tensor_sub(Me, V, Se)
                    nc.scalar.dma_start(
                        out_d[dsl, q * OTW + 3 * UO : (q + 1) * OTW],
                        ot[:, 3 * UO : OTW],
                    )
                else:
                    nc.vector.tensor_sub(Me, V, Se)
                    nc.scalar.dma_start(
                        out_d[dsl, q * OTW : (q + 1) * OTW], ot[:]
                    )

            units = [(q, di) for q in range(NQ) for di in range(4)]
            prev = None
            mcall = 0
            for u in units:
                q, di = u
                if di == 0:
                    xe_ts, xo_ts = [], []
                    qc0 = q * UO
                    for k, (r0_, rk) in enumerate(KT):
                        te = xp.tile([128, UO], FP16, tag=f"xe{k}", name=f"xe{k}_{q}")
                        nc.sync.dma_start(
                            te[:rk, :], xe_d[r0_ : r0_ + rk, qc0 : qc0 + UO]
                        )
                        xe_ts.append(te)
                        to = xp.tile([128, UO], FP16, tag=f"xo{k}", name=f"xo{k}_{q}")
                        # q0's xo rides the otherwise-idle gpsimd queue so
                        # the first quartet lands on two queues in parallel
                        (nc.gpsimd if q == 0 else nc.sync).dma_start(
                            to[:rk, :], xo_d[r0_ : r0_ + rk, qc0 : qc0 + UO]
                        )
                        xo_ts.append(to)
                    state["x"] = (xe_ts, xo_ts)
                    if q == 0:
                        # cf1/cf2 queue behind q0's x on sync/gpsimd
                        nc.sync.dma_start(cf_t[1][:], cf_d[1])
                        nc.gpsimd.dma_start(cf_t[2][:], cf_d[2])
                syn_part(u)
                if prev is not None:
                    mem_part(prev, mcall)
                    mcall += 1
                prev = u
            mem_part(prev, mcall)

    nc.compile()
    _cache[key] = nc
    return nc


def kernel(inputs, w, alpha, beta):
    global LAST_RESULT
    inputs = np.asarray(inputs, dtype=np.float32)
    w = np.asarray(w, dtype=np.float32)
    alpha = np.asarray(alpha, dtype=np.float32).reshape(-1)
    beta = np.asarray(beta, dtype=np.float32).reshape(-1)

    nc = _build()

    def coef(sq):
        c = np.broadcast_to(sq.reshape(4, 128, 1), (4, 128, UO)).astype(np.float32).copy()
        c3 = c.reshape(4, 128, 16, K)
        c3[:, :, :, 0] = 0.0
        return c

    coefs = np.concatenate(
        [
            coef(alpha * alpha),
            coef(beta * beta),
            (alpha * beta).reshape(4, 128, 1).astype(np.float32),
        ],
        axis=2,
    )
    wpack = np.zeros((128, 12 * D), dtype=np.float16)
    wa = (w * alpha.reshape(1, D)).astype(np.float16)
    w16f = w.astype(np.float16)
    for k, (r0_, rk) in enumerate(KT):
        wpack[:rk, k * D : k * D + D] = w16f[r0_ : r0_ + rk, :]
        wpack[:rk, 6 * D + k * D : 6 * D + k * D + D] = wa[r0_ : r0_ + rk, :]

    in_maps = []
    for c in range(NCORES):
        xc = inputs[c * BC : (c + 1) * BC]  # [64, 100, 700]
        xe = xc[:, 0::2, :]  # [64, 50, 700] (t = 0,2,..,98)
        xo = np.zeros((BC, K, C), dtype=np.float32)
        xo[:, 1:, :] = xc[:, 1:98:2, :]  # t = 1,3,..,97 -> k=1..49
        xe16 = xe.reshape(BC * K, C).T.astype(np.float16).copy()
        xo16 = xo.reshape(BC * K, C).T.astype(np.float16).copy()
        in_maps.append(
            {
                "xe16": xe16,
                "xo16": xo16,
                "w16": wpack,
                "coefs": coefs,
            }
        )

    run_kwargs = {}
    if os.environ.get("MEMBRANE_TRACE_DIR"):
        run_kwargs["tmpdir"] = os.environ["MEMBRANE_TRACE_DIR"]
    res = bass_utils.run_bass_kernel_spmd(
        nc, in_maps, core_ids=list(range(NCORES)), **run_kwargs
    )
    LAST_RESULT = res

    inv_a = (1.0 / alpha).reshape(1, 1, D)
    mo_sc = ((1.0 - beta) / alpha).reshape(1, 1, D)
    me_sc = ((1.0 - beta) / (alpha * beta)).reshape(1, 1, D)

    syn_full = np.empty((B, T, D), dtype=np.float32)
    mem_full = np.empty((B, T, D), dtype=np.float32)
    for c in range(NCORES):
        r = res.results[c]["out"].astype(np.float32)  # [512, NQ*OTW]
        for q in range(NQ):
            blk = r[:, q * OTW : (q + 1) * OTW]
            so = blk[:, 0:UO].reshape(D, 16, K)  # t=2k+1
            V = blk[:, UO : 2 * UO].reshape(D, 16, K)
            Sev = blk[:, 2 * UO : 3 * UO].reshape(D, 16, K)
            Mev = blk[:, 3 * UO : OTW].reshape(D, 16, K)
            b0 = c * BC + q * 16
            syn_full[b0 : b0 + 16, 1::2, :] = so.transpose(1, 2, 0)
            syn_full[b0 : b0 + 16, 0::2, :] = Sev.transpose(1, 2, 0) * inv_a
            mem_full[b0 : b0 + 16, 1::2, :] = V.transpose(1, 2, 0) * mo_sc
            mem_full[b0 : b0 + 16, 0::2, :] = Mev.transpose(1, 2, 0) * me_sc
    return (syn_full, mem_full)
